# revision 67
# baseline (speedup 1.0000x reference)
"""Trainium2 Bass kernel for the Chebyshev atomic descriptor (gnn_message_passing).

Contract: kernel(**inputs) takes FULL unsharded inputs (positions [20000,3] f32,
species_idx [20000] i32, neighbor_idx [480000] i32) and returns the full
[20000, 52] f32 feature array. Internally shards atoms across 8 NeuronCores
(data-parallel over N) and gathers neighbor rows on-device via indirect DMA.

Algorithm: the angular (triplet) features are computed via the spherical-
harmonic addition theorem instead of the O(K^2) pair sum:
  sum_{j<k} w_j w_k T_t(u_j.u_k) = 1/2 (sum_l lam_{t,l} Q_l - F2),
  Q_l = sum_m gamma_lm B_lm^2,  B_lm = sum_j w_j Ybar_lm(u_j),  F2 = sum_j w_j^2
with real solid harmonics evaluated per neighbor by sectoral (x+iy)^m and
associated-Legendre z-ladder recurrences in fp16. All reductions over the K=24
neighbors (angular moments, radial Chebyshev chains, F2) run on the otherwise
idle TensorEngine as identity-stationary matmuls accumulating in PSUM.
"""

import math
from contextlib import ExitStack

import numpy as np

import bass_rust
import concourse.bass as bass
import concourse.bacc as bacc
import concourse.tile as tile
from concourse import mybir
from concourse.bass_utils import run_bass_kernel_spmd

F32 = mybir.dt.float32
F16 = mybir.dt.float16
I32 = mybir.dt.int32
Alu = mybir.AluOpType
Act = mybir.ActivationFunctionType
AX = mybir.AxisListType

# ---- problem constants (hardcoded per harness contract) ----
N = 20000
K = 24
NCORES = 8
NPAD = 20480
NPC = NPAD // NCORES     # atoms per core = 2560
PT = 128                 # partitions
G = 5                    # atoms per partition per supertile
SUP = NPC // (PT * G)    # supertiles per core = 4
STA = PT * G             # atoms per supertile = 640
SLOT = G * K             # neighbor slots per partition per supertile = 120
RAD_ORDER = 16
ANG_ORDER = 8
L = ANG_ORDER
NRAD = RAD_ORDER + 1     # 17
NANG = ANG_ORDER + 1     # 9
RAD_CUT = 8.0
ANG_CUT = 6.5
MIN_CUT = 0.55
FEAT = 52
NC_RECT = 9 * 9 * 2      # 162 rect comps (l, m, trig)
NRADC = NRAD + 1         # radial comps + F2 slot = 18

HALF_PI = math.pi / 2.0
AX_ = 2.0 / (RAD_CUT - MIN_CUT)
BX_ = -2.0 * MIN_CUT / (RAD_CUT - MIN_CUT) - 1.0

ROWE = 64                # gather table row: 64 f32 = 256B (dma_gather granularity)
GQ = 2                   # dma_gather calls per supertile
CQ = SLOT // GQ          # gathered slots per partition per call = 60
NIDX = CQ * PT           # indices per gather call = 7680


# ---------------------------------------------------------------------------
# host-side constant tables (ladder recurrence + quadratic-form weights)
# ---------------------------------------------------------------------------
def _dfact(n):
    r = 1
    while n > 1:
        r *= n
        n -= 2
    return r


def _a_norm(l, m):
    if m == 0:
        return 1.0
    return math.sqrt(2.0 * math.factorial(l - m) / math.factorial(l + m))


def _ladder_coeffs():
    """Monic z-ladder: A~_m = 1, A~_{m+1} = z, A~_l = z A~_{l-1} + gt A~_{l-2};
    Ybar_lm = sig_lm * A~_lm * trig_m. Returns gt[(l,m)], sig[(l,m)]."""
    gt, sig = {}, {}
    for m in range(L + 1):
        k = {m: 1.0 / _dfact(2 * m - 1)}
        if m + 1 <= L:
            k[m + 1] = k[m] / (2 * m + 1)
        for l in range(m + 2, L + 1):
            beta = (2 * l - 1) / (l - m)
            gam = -(l + m - 1) / (l - m)
            k[l] = k[l - 1] / beta
            gt[(l, m)] = gam * k[l] / k[l - 2]
        for l in range(m, L + 1):
            sig[(l, m)] = _a_norm(l, m) / k[l]
    return gt, sig


def _cheb_to_legendre():
    from numpy.polynomial import legendre as npleg, chebyshev as npcheb

    lam = np.zeros((NANG, L + 1))
    for t in range(NANG):
        c = np.zeros(t + 1)
        c[t] = 1.0
        lam[t, : t + 1] = npleg.poly2leg(npcheb.cheb2poly(c))[: t + 1]
    return lam


LAM = _cheb_to_legendre()
GT, SIG = _ladder_coeffs()


def _const_tables():
    # ccoef f16 [81]: gt at slot l*9+m (l-major), 0 elsewhere
    ccoef = np.zeros(81, np.float16)
    for (l, m), v in GT.items():
        ccoef[l * 9 + m] = np.float16(v)
    # gam f32 [162]: sig^2 at rect slot (l*9+m)*2+t for valid (m<=l), else 0
    gam = np.zeros(NC_RECT, np.float32)
    for l in range(L + 1):
        for m in range(l + 1):
            g = np.float32(SIG[(l, m)]) ** 2
            gam[(l * 9 + m) * 2 + 0] = g
            if m >= 1:
                gam[(l * 9 + m) * 2 + 1] = g
    ident = np.eye(PT, dtype=np.float16)
    return ccoef, gam, ident


def view(ap, off, dims):
    """Free-dim view of a tile AP: keep the partition entry, replace free dims
    with explicit [step, count] pairs, shift the element offset by `off`."""
    base = list(ap.ap[0])
    return bass_rust.AP(ap.tensor, ap.offset + off, [base] + [list(d) for d in dims])


def build_supertile(nc, ctx, s, tl, pself, feat_dram, mix_prev=None):
    """Emit one supertile's compute. tl = dict of persistent tiles.
    mix_prev: emitted on DVE between the ladder and weights phases — fills the
    engine while this supertile waits on sectoral (Pool) and the previous
    supertile's matmuls (PE)."""
    base = s * STA

    pn = tl[f"pn{s % 2}"]
    ps = tl[f"ps{s % 2}"]

    # ---- prep + radial + sectoral, emitted per slot-range (lo, n) so the
    # first supertile can start on the first gather call's half ----
    half_pi = tl["half_pi"]
    Srad = tl["Srad"]
    SEC = tl["SEC"]
    uz = tl["uz"]

    def prep_range(lo, n):
        r012 = tl["r012"]
        r_c = [view(r012[:], c * SLOT + lo, [[1, n]]) for c in range(3)]
        for c in range(3):
            nc.vector.tensor_tensor(
                out=r_c[c],
                in0=view(pn[:], c + 4 * lo, [[4, n]]),
                in1=view(ps[:], c, [[0, n // G], [4, G]]),
                op=Alu.subtract,
            )
        sq012 = tl["sq012"]
        sq = [view(sq012[:], c * SLOT + lo, [[1, n]]) for c in range(3)]
        for c in range(3):
            nc.scalar.activation(sq[c], r_c[c], Act.Square)
        d2 = view(tl["d2"][:], lo, [[1, n]])
        nc.vector.tensor_tensor(out=d2, in0=sq[0], in1=sq[1], op=Alu.add)
        nc.vector.tensor_tensor(out=d2, in0=d2, in1=sq[2], op=Alu.add)
        nc.vector.tensor_scalar_max(d2, d2, 1e-18)
        dd = view(tl["dd"][:], lo, [[1, n]])
        nc.scalar.sqrt(dd, d2)
        rinv = view(tl["rinv"][:], lo, [[1, n]])
        nc.vector.reciprocal(rinv, dd)

        # unit vector: x,y straight into SEC block m=1; z separate (f16)
        nc.vector.tensor_tensor(
            out=view(SEC[:], lo, [[1, n]]), in0=r_c[0], in1=rinv, op=Alu.mult
        )
        nc.vector.tensor_tensor(
            out=view(SEC[:], SLOT + lo, [[1, n]]), in0=r_c[1], in1=rinv, op=Alu.mult
        )
        nc.vector.tensor_tensor(
            out=view(uz[:], lo, [[1, n]]), in0=r_c[2], in1=rinv, op=Alu.mult
        )

        m2 = view(tl["m2"][:], lo, [[1, n]])
        nc.vector.tensor_scalar(
            out=m2, in0=dd, scalar1=MIN_CUT, scalar2=None, op0=Alu.is_gt
        )
        # radial weight wr = fc * mask  (wr = mh*(grad+1), mh = 0.5*mask)
        dcr = view(tl["dcr"][:], lo, [[1, n]])
        nc.vector.tensor_scalar_min(dcr, dd, RAD_CUT)
        grad = view(tl["grad"][:], lo, [[1, n]])
        nc.scalar.activation(
            grad, dcr, Act.Sin, bias=half_pi[:], scale=-math.pi / RAD_CUT
        )
        m1h = view(tl["m1h"][:], lo, [[1, n]])
        nc.vector.tensor_scalar(
            out=m1h, in0=dd, scalar1=RAD_CUT, scalar2=0.5, op0=Alu.is_le, op1=Alu.mult
        )
        nc.vector.tensor_tensor(out=m1h, in0=m1h, in1=m2, op=Alu.mult)
        # S0 = wr (f16), written directly into the radial chain tile
        nc.vector.scalar_tensor_tensor(
            out=view(Srad[:], lo, [[1, n]]),
            in0=grad,
            scalar=1.0,
            in1=m1h,
            op0=Alu.add,
            op1=Alu.mult,
        )
        # angular weight w = fca * mask
        dca = view(tl["dcr"][:], lo, [[1, n]])
        nc.vector.tensor_scalar_min(dca, dd, ANG_CUT)
        gang = view(tl["gang"][:], lo, [[1, n]])
        nc.scalar.activation(
            gang, dca, Act.Sin, bias=half_pi[:], scale=-math.pi / ANG_CUT
        )
        a1h = view(tl["a1h"][:], lo, [[1, n]])
        nc.vector.tensor_scalar(
            out=a1h, in0=dd, scalar1=ANG_CUT, scalar2=0.5, op0=Alu.is_le, op1=Alu.mult
        )
        nc.vector.tensor_tensor(out=a1h, in0=a1h, in1=m2, op=Alu.mult)
        wh = view(tl["wh"][:], lo, [[1, n]])
        nc.vector.scalar_tensor_tensor(
            out=wh, in0=gang, scalar=1.0, in1=a1h, op0=Alu.add, op1=Alu.mult
        )
        snh = view(tl["snh"][:], lo, [[1, n]])
        nc.scalar.copy(snh, view(pn[:], 3 + 4 * lo, [[4, n]]))
        nc.vector.tensor_tensor(
            out=view(tl["wsh"][:], lo, [[1, n]]), in0=wh, in1=snh, op=Alu.mult
        )
        # radial chebyshev argument (f16)
        nc.scalar.activation(
            view(tl["xxh"][:], lo, [[1, n]]), dd, Act.Copy, bias=BX_, scale=AX_
        )
        nc.scalar.activation(
            view(tl["x2h"][:], lo, [[1, n]]), dd, Act.Copy, bias=2 * BX_, scale=2 * AX_
        )

    def radial_range(lo, n):
        xxh, x2h, wh, snh = tl["xxh"], tl["x2h"], tl["wh"], tl["snh"]
        nc.vector.tensor_tensor(
            out=view(Srad[:], SLOT + lo, [[1, n]]),
            in0=view(xxh[:], lo, [[1, n]]),
            in1=view(Srad[:], lo, [[1, n]]),
            op=Alu.mult,
        )
        rtmp = view(tl["rtmp"][:], lo, [[1, n]])
        for t in range(2, NRAD):
            nc.vector.tensor_tensor(
                out=rtmp,
                in0=view(x2h[:], lo, [[1, n]]),
                in1=view(Srad[:], (t - 1) * SLOT + lo, [[1, n]]),
                op=Alu.mult,
            )
            nc.vector.tensor_tensor(
                out=view(Srad[:], t * SLOT + lo, [[1, n]]),
                in0=rtmp,
                in1=view(Srad[:], (t - 2) * SLOT + lo, [[1, n]]),
                op=Alu.subtract,
            )
        # F2 = w^2 appended as radial comp 17 (chain A)
        nc.vector.tensor_tensor(
            out=view(Srad[:], NRAD * SLOT + lo, [[1, n]]),
            in0=view(wh[:], lo, [[1, n]]),
            in1=view(wh[:], lo, [[1, n]]),
            op=Alu.mult,
        )
        # chain B: spin-weighted radial values, comps 18..34
        nc.vector.tensor_tensor(
            out=view(Srad[:], NRADC * SLOT + lo, [[SLOT, NRAD], [1, n]]),
            in0=view(Srad[:], lo, [[SLOT, NRAD], [1, n]]),
            in1=view(snh[:], lo, [[0, NRAD], [1, n]]),
            op=Alu.mult,
        )

    def sectoral_range(lo, n):
        tc_ = view(tl["tc_"][:], lo, [[1, n]])
        td_ = view(tl["td_"][:], lo, [[1, n]])
        ux_v = view(SEC[:], lo, [[1, n]])
        uy_v = view(SEC[:], SLOT + lo, [[1, n]])
        for m in range(2, L + 1):
            cp = (m - 2) * 2 * SLOT + lo
            sp = cp + SLOT
            cm = (m - 1) * 2 * SLOT + lo
            sm = cm + SLOT
            nc.gpsimd.tensor_tensor(out=tc_, in0=ux_v, in1=view(SEC[:], cp, [[1, n]]), op=Alu.mult)
            nc.gpsimd.tensor_tensor(out=td_, in0=uy_v, in1=view(SEC[:], sp, [[1, n]]), op=Alu.mult)
            nc.gpsimd.tensor_tensor(
                out=view(SEC[:], cm, [[1, n]]), in0=tc_, in1=td_, op=Alu.subtract
            )
            nc.gpsimd.tensor_tensor(out=tc_, in0=ux_v, in1=view(SEC[:], sp, [[1, n]]), op=Alu.mult)
            nc.gpsimd.tensor_tensor(out=td_, in0=uy_v, in1=view(SEC[:], cp, [[1, n]]), op=Alu.mult)
            nc.gpsimd.tensor_tensor(
                out=view(SEC[:], sm, [[1, n]]), in0=tc_, in1=td_, op=Alu.add
            )

    if s == 0:
        # supertile 0: prep per gather-half so compute starts on the first call
        prep_range(0, CQ)
        prep_range(CQ, CQ)
        radial_range(0, SLOT)
        sectoral_range(0, SLOT)
    else:
        prep_range(0, SLOT)
        radial_range(0, SLOT)
        sectoral_range(0, SLOT)

    # ---- z-ladder (f16, l-major LAD: slot (l*9+m)*SLOT) ----
    LAD = tl["LAD"]
    ccoef = tl["ccoef"]
    # l = m+1 diagonal row: A~_{m+1,m} = z for m=0..7 (slots m*10+9)
    nc.vector.tensor_copy(
        out=view(LAD[:], 9 * SLOT, [[10 * SLOT, 8], [1, SLOT]]),
        in_=view(uz[:], 0, [[0, 8], [1, SLOT]]),
    )
    lt = tl["lt"]
    for l in range(2, L + 1):
        nm = l - 1  # m = 0..l-2
        nc.vector.tensor_tensor(
            out=view(LAD[:], l * 9 * SLOT, [[SLOT, nm], [1, SLOT]]),
            in0=view(uz[:], 0, [[0, nm], [1, SLOT]]),
            in1=view(LAD[:], (l - 1) * 9 * SLOT, [[SLOT, nm], [1, SLOT]]),
            op=Alu.mult,
        )
        nc.vector.tensor_tensor(
            out=view(lt[:], 0, [[SLOT, nm], [1, SLOT]]),
            in0=view(ccoef[:], l * 9, [[1, nm], [0, SLOT]]),
            in1=view(LAD[:], (l - 2) * 9 * SLOT, [[SLOT, nm], [1, SLOT]]),
            op=Alu.mult,
        )
        nc.vector.tensor_tensor(
            out=view(LAD[:], l * 9 * SLOT, [[SLOT, nm], [1, SLOT]]),
            in0=view(LAD[:], l * 9 * SLOT, [[SLOT, nm], [1, SLOT]]),
            in1=view(lt[:], 0, [[SLOT, nm], [1, SLOT]]),
            op=Alu.add,
        )

    # ---- weight tiles WA/WB (f16): (m, trig) slots ----
    WA, WB = tl["WA"], tl["WB"]
    nc.vector.tensor_copy(out=view(WA[:], 0, [[1, SLOT]]), in_=tl["wh"][:])
    nc.vector.tensor_copy(out=view(WB[:], 0, [[1, SLOT]]), in_=tl["wsh"][:])
    for m in range(1, L + 1):
        sec_b = view(SEC[:], (m - 1) * 2 * SLOT, [[SLOT, 2], [1, SLOT]])
        nc.vector.tensor_tensor(
            out=view(WA[:], m * 2 * SLOT, [[SLOT, 2], [1, SLOT]]),
            in0=view(tl["wh"][:], 0, [[0, 2], [1, SLOT]]),
            in1=sec_b,
            op=Alu.mult,
        )
        nc.vector.tensor_tensor(
            out=view(WB[:], m * 2 * SLOT, [[SLOT, 2], [1, SLOT]]),
            in0=view(tl["wsh"][:], 0, [[0, 2], [1, SLOT]]),
            in1=sec_b,
            op=Alu.mult,
        )

    # ---- products into MP rect (f16): MP[(l*9+m)*2+t] = W[m,t] * A~[l,m] ----
    # m=0 has no sin comp: single-trig product; its sin slots are zeroed once.
    for chain, W in enumerate((WA, WB)):
        MP = tl[f"MP{(2 * s + chain) % 3}"]
        nc.vector.tensor_tensor(
            out=view(MP[:], 0, [[18 * SLOT, 9], [1, SLOT]]),
            in0=view(W[:], 0, [[0, 9], [1, SLOT]]),
            in1=view(LAD[:], 0, [[9 * SLOT, 9], [1, SLOT]]),
            op=Alu.mult,
        )
        for m in range(1, L + 1):
            nl = 9 - m
            nc.vector.tensor_tensor(
                out=view(MP[:], m * 20 * SLOT, [[18 * SLOT, nl], [SLOT, 2], [1, SLOT]]),
                in0=view(W[:], m * 2 * SLOT, [[0, nl], [SLOT, 2], [1, SLOT]]),
                in1=view(LAD[:], m * 10 * SLOT, [[9 * SLOT, nl], [0, 2], [1, SLOT]]),
                op=Alu.mult,
            )

    if mix_prev is not None:
        mix_prev()

    # ---- K-reduction on PE: identity-stationary accumulating matmuls ----
    # out per matmul must stay inside one PSUM bank (512 f32): split the 162
    # rect comps into two 81-comp groups per chain at bank-aligned offsets.
    # Radial first so the next supertile's radial chain unblocks earliest.
    ident = tl["ident"]
    accA, accB, accR = tl["accA"], tl["accB"], tl["accR"]
    HC = NC_RECT // 2  # 81
    featt = tl["featt"]
    foff = (s % 2) * G * FEAT
    SQ = tl["SQ"]
    gam = tl["gam"]
    Q = tl["Q"]

    for k in range(K):
        nc.tensor.matmul(
            view(accR[:], 0, [[1, (NRADC + NRAD) * G]]),
            ident[:],
            view(Srad[:], k * G, [[SLOT, NRADC + NRAD], [1, G]]),
            start=(k == 0),
            stop=(k == K - 1),
        )
    for ci, acc in enumerate((accA, accB)):
        MP = tl[f"MP{(2 * s + ci) % 3}"]
        for half in range(2):
            for k in range(K):
                nc.tensor.matmul(
                    view(acc[:], half * 512, [[1, HC * G]]),
                    ident[:],
                    view(MP[:], half * HC * SLOT + k * G, [[SLOT, HC], [1, G]]),
                    start=(k == 0),
                    stop=(k == K - 1),
                )


def build_mix(nc, tl, s, feat_dram):
    """B^2 evac + gamma-weight + Q-reduce + lambda-mix + store for supertile s
    (emitted later, while a following supertile's matmuls occupy the PE, so
    the ACT/DVE queues don't stall the next supertile's prep)."""
    SQ, gam, Q, featt = tl["SQ"], tl["gam"], tl["Q"], tl["featt"]
    HC = NC_RECT // 2
    foff = (s % 2) * G * FEAT
    accR = tl["accR"]
    # rad_un (f 0..16) and rad_w (f 17..33): iter (t, g) -> featt[g*52 + f]
    nc.scalar.copy(
        out=view(featt[:], foff + 0, [[1, NRAD], [FEAT, G]]),
        in_=view(accR[:], 0, [[G, NRAD], [1, G]]),
    )
    nc.scalar.copy(
        out=view(featt[:], foff + NRAD, [[1, NRAD], [FEAT, G]]),
        in_=view(accR[:], NRADC * G, [[G, NRAD], [1, G]]),
    )
    # F2 (radial comp 17 of chain A) -> F2S[s]
    nc.scalar.copy(
        out=view(tl["F2S"][:], (s % 2) * G, [[1, G]]),
        in_=view(accR[:], NRAD * G, [[1, G]]),
    )
    for ci, acc in enumerate((tl["accA"], tl["accB"])):
        soff = ci * NC_RECT * G
        for half in range(2):
            nc.scalar.activation(
                view(SQ[:], soff + half * HC * G, [[1, HC * G]]),
                view(acc[:], half * 512, [[1, HC * G]]),
                Act.Square,
            )
        nc.vector.tensor_tensor(
            out=view(SQ[:], soff, [[1, NC_RECT * G]]),
            in0=view(SQ[:], soff, [[1, NC_RECT * G]]),
            in1=view(gam[:], 0, [[1, NC_RECT], [0, G]]),
            op=Alu.mult,
        )
        nc.vector.tensor_reduce(
            out=view(Q[:], ((s % 2) * 2 + ci) * 9 * G, [[G, 9], [1, G]]),
            in_=view(SQ[:], soff, [[18 * G, 9], [1, G], [G, 18]]),
            axis=AX.X,
            op=Alu.add,
        )
    # ang[t] = sum_l 0.5 lam[t,l] Q_l - 0.5 F2
    F2h = tl["F2h"]
    nc.vector.tensor_scalar(
        out=view(F2h[:], (s % 2) * G, [[1, G]]),
        in0=view(tl["F2S"][:], (s % 2) * G, [[1, G]]),
        scalar1=0.5,
        scalar2=None,
        op0=Alu.mult,
    )
    mixa, mixb = tl["mixa"], tl["mixb"]
    for chain in range(2):
        fbase = 2 * NRAD + chain * NANG
        for t in range(NANG):
            ls = list(range(t % 2, t + 1, 2))
            acc = None
            for i, l in enumerate(ls):
                qv = view(Q[:], ((s % 2) * 2 + chain) * 9 * G + l * G, [[1, G]])
                lam = 0.5 * float(LAM[t, l])
                last = i == len(ls) - 1
                dst = (
                    view(featt[:], foff + fbase + t, [[FEAT, G]])
                    if last
                    else view((mixb if acc is mixa else mixa)[:], 0, [[1, G]])
                )
                if i == 0:
                    src = view(F2h[:], (s % 2) * G, [[1, G]])
                    op1 = Alu.subtract
                else:
                    src = view((mixa if acc is mixa else mixb)[:], 0, [[1, G]])
                    op1 = Alu.add
                nc.vector.scalar_tensor_tensor(
                    out=dst, in0=qv, scalar=lam, in1=src, op0=Alu.mult, op1=op1
                )
                acc = mixa if (acc is not mixa) else mixb

    nc.sync.dma_start(
        out=feat_dram[s * STA : (s + 1) * STA, :].rearrange("(p g) f -> p (g f)", p=PT),
        in_=view(featt[:], foff, [[1, G * FEAT]]),
    )


def build_program():
    I16 = mybir.dt.int16
    nc = bacc.Bacc("TRN2", target_bir_lowering=False, debug=False)
    pos4 = nc.dram_tensor("pos4", [NPAD, ROWE], F32, kind="ExternalInput").ap()
    idx = nc.dram_tensor(
        "idx", [SUP * GQ * PT, NIDX // 16], I16, kind="ExternalInput"
    ).ap()
    pself = nc.dram_tensor("pself", [NPC, 4], F32, kind="ExternalInput").ap()
    ident_d = nc.dram_tensor("ident", [PT, PT], F16, kind="ExternalInput").ap()
    ccoef_d = nc.dram_tensor("ccoef", [PT, 81], F16, kind="ExternalInput").ap()
    gam_d = nc.dram_tensor("gam", [PT, NC_RECT], F32, kind="ExternalInput").ap()
    feat = nc.dram_tensor("feat", [NPC, FEAT], F32, kind="ExternalOutput").ap()

    with tile.TileContext(nc) as tc, ExitStack() as ctx:
        const = ctx.enter_context(tc.tile_pool(name="const", bufs=1))
        io = ctx.enter_context(tc.tile_pool(name="io", bufs=1))
        kp = ctx.enter_context(tc.tile_pool(name="kspace", bufs=1))
        psum = ctx.enter_context(tc.tile_pool(name="psum", bufs=1, space="PSUM"))

        tl = {}

        def T(pool, name, shape, dtype):
            tl[name] = pool.tile(shape, dtype, name=name, tag=name)
            return tl[name]

        # constants
        T(const, "ident", [PT, PT], F16)
        T(const, "ccoef", [PT, 81], F16)
        T(const, "gam", [PT, NC_RECT], F32)
        T(const, "half_pi", [PT, 1], F32)

        # io (double-buffered via explicit 0/1 tiles; single pnw stage)
        I16 = mybir.dt.int16
        for b in range(2):
            T(io, f"pn{b}", [PT, SLOT * 4], F32)
            T(io, f"ps{b}", [PT, G * 4], F32)
        for b in range(2):
            T(io, f"pnw{b}", [PT, CQ * ROWE], F32)
            T(io, f"idxt{b}", [PT, NIDX // 16], I16)


        # prep f32
        for nm in ("d2", "dd", "rinv", "m2", "dcr", "grad", "m1h", "gang", "a1h"):
            T(kp, nm, [PT, SLOT], F32)
        T(kp, "r012", [PT, 3 * SLOT], F32)
        T(kp, "sq012", [PT, 3 * SLOT], F32)
        # f16 working set
        for nm in ("uz", "wh", "snh", "wsh", "xxh", "x2h", "rtmp", "tc_", "td_"):
            T(kp, nm, [PT, SLOT], F16)
        T(kp, "SEC", [PT, 8 * 2 * SLOT], F16)
        T(kp, "LAD", [PT, 81 * SLOT], F16)
        T(kp, "lt", [PT, 7 * SLOT], F16)
        T(kp, "WA", [PT, NC_RECT // 9 * SLOT], F16)
        T(kp, "WB", [PT, NC_RECT // 9 * SLOT], F16)
        for b in range(3):
            T(kp, f"MP{b}", [PT, NC_RECT * SLOT], F16)
        T(kp, "Srad", [PT, (NRADC + NRAD) * SLOT], F16)
        T(kp, "SQ", [PT, 2 * NC_RECT * G], F16)
        T(kp, "featt", [PT, 2 * G * FEAT], F32)
        T(kp, "F2S", [PT, 2 * G], F32)
        T(kp, "F2h", [PT, 2 * G], F32)
        T(kp, "Q", [PT, 2 * 2 * 9 * G], F32)
        T(kp, "mixa", [PT, SUP * G], F32)
        T(kp, "mixb", [PT, SUP * G], F32)

        # psum accumulators (bank-padded: each matmul target inside one bank)
        T(psum, "accA", [PT, 1024], F32)
        T(psum, "accB", [PT, 1024], F32)
        T(psum, "accR", [PT, 512], F32)

        def memset_mp(MPn):
            # zero invalid MP slots (m > l) and the nonexistent m=0 sin comps
            MP = tl[MPn]
            for m in range(1, L + 1):
                nc.gpsimd.memset(
                    view(MP[:], m * 2 * SLOT, [[18 * SLOT, m], [1, 2 * SLOT]]), 0.0
                )
            nc.gpsimd.memset(view(MP[:], SLOT, [[18 * SLOT, 9], [1, SLOT]]), 0.0)

        def emit_onetime_memsets():
            memset_mp("MP0")
            memset_mp("MP1")
            nc.gpsimd.memset(
                view(tl["LAD"][:], 0, [[10 * SLOT, 9], [1, SLOT]]), 1.0
            )

        def gather(s):
            b = s % 2
            pn = tl[f"pn{b}"]
            for q in range(GQ):
                qb = (s * GQ + q) % 2
                idxt = tl[f"idxt{qb}"]
                pnw = tl[f"pnw{qb}"]
                row0 = (s * GQ + q) * PT
                nc.sync.dma_start(out=idxt[:], in_=idx[row0 : row0 + PT, :])
                nc.gpsimd.dma_gather(
                    out_ap=view(pnw[:], 0, [[ROWE, CQ], [1, ROWE]]),
                    in_ap=pos4,
                    idxs_ap=idxt[:],
                    num_idxs=NIDX,
                    num_idxs_reg=NIDX,
                    elem_size=ROWE,
                    single_packet=False,
                )
                nc.scalar.copy(
                    out=view(pn[:], q * CQ * 4, [[1, CQ * 4]]),
                    in_=view(pnw[:], 0, [[ROWE, CQ], [1, 4]]),
                )
            nc.sync.dma_start(
                out=tl[f"ps{b}"][:],
                in_=pself[s * STA : (s + 1) * STA, :].rearrange(
                    "(p g) c -> p (g c)", p=PT
                ),
            )

        gather(0)
        nc.sync.dma_start(out=tl["ident"][:], in_=ident_d)
        nc.sync.dma_start(out=tl["ccoef"][:], in_=ccoef_d)
        nc.sync.dma_start(out=tl["gam"][:], in_=gam_d)
        nc.gpsimd.memset(tl["half_pi"][:], HALF_PI)
        emit_onetime_memsets()
        for s in range(SUP):
            if s == 1:
                memset_mp("MP2")
            if s + 1 < SUP:
                gather(s + 1)
            mix_prev = (
                (lambda sp=s - 1: build_mix(nc, tl, sp, feat)) if s > 0 else None
            )
            build_supertile(nc, ctx, s, tl, pself, feat, mix_prev=mix_prev)
        build_mix(nc, tl, SUP - 1, feat)

    nc.compile()
    return nc


_NC_CACHE = None


def get_program():
    global _NC_CACHE
    if _NC_CACHE is None:
        _NC_CACHE = build_program()
    return _NC_CACHE


def make_in_maps(positions, species_idx, neighbor_idx):
    pos4 = np.zeros((NPAD, ROWE), np.float32)
    pos4[:N, :3] = positions
    pos4[:N, 3] = 2.0 * species_idx.astype(np.float32) - 1.0
    nbrK = np.zeros((NPAD, K), np.int32)
    nbrK[:N] = neighbor_idx.reshape(N, K)

    ccoef, gam, ident = _const_tables()
    ccoef_t = np.broadcast_to(ccoef, (PT, 81)).copy()
    gam_t = np.broadcast_to(gam, (PT, NC_RECT)).copy()

    c_idx = np.arange(SLOT)
    k_of, g_of = c_idx // G, c_idx % G
    p = np.arange(PT)
    in_maps = []
    for c in range(NCORES):
        cb = c * NPC
        blocks = []
        for s in range(SUP):
            # vals[slot, p] = nbrK[cb + s*STA + p*G + g(slot), k(slot)]
            atoms = cb + s * STA + p[None, :] * G + g_of[:, None]  # [SLOT, PT]
            vals = nbrK[atoms, k_of[:, None]].astype(np.int16)
            for q in range(GQ):
                flat = vals[q * CQ : (q + 1) * CQ, :].reshape(-1)  # i = cc*128+p
                wrapped = flat.reshape(-1, 16).T  # [16, NIDX/16]
                blocks.append(np.tile(wrapped, (PT // 16, 1)))
        idx16 = np.concatenate(blocks, axis=0)  # [SUP*GQ*PT, NIDX/16]
        in_maps.append(
            {
                "pos4": pos4,
                "idx": np.ascontiguousarray(idx16),
                "pself": np.ascontiguousarray(pos4[cb : cb + NPC, :4]),
                "ident": ident,
                "ccoef": ccoef_t,
                "gam": gam_t,
            }
        )
    return in_maps


def run(positions, species_idx, neighbor_idx, trace=False, trace_cores=None):
    nc = get_program()
    in_maps = make_in_maps(positions, species_idx, neighbor_idx)
    res = run_bass_kernel_spmd(
        nc,
        in_maps,
        core_ids=list(range(NCORES)),
        trace=trace,
        trace_cores=trace_cores,
    )
    out = np.concatenate([res.results[c]["feat"] for c in range(NCORES)], axis=0)
    return out[:N], res


def kernel(positions, species_idx, neighbor_idx):
    out, _ = run(positions, species_idx, neighbor_idx, trace=False)
    return out
